# revision 41
# baseline (speedup 1.0000x reference)
"""Trainium2 Bass kernel for nn_CE_55937654063537.

Reference computation:
    b1 = conv3x3(x, g_w) + g_b            [B, 2, 512, 512]
    b2 = conv1x1(x, theta_w) + theta_b    [B, 2, 512, 512]
    m  = patch_mean(b1, 7) + patch_mean(b2, 7)   [B, 2, 7, 7]
    out = bilinear_upsample(m, 512, 512)  (half-pixel centers)

Everything is linear, so the kernel never materializes the conv outputs.
patch_mean(feat)[i, j] is (1/(H*W)) * the sum of feat over a rectangle that is
the full map minus <=3 boundary rows/cols.  Those rectangle sums are linear in
(a) the column-sum over h of x and (b) 8 boundary rows of x.

The kernel is HBM-streaming bound.  Trace-driven schedule:
  * input is streamed as one 4 MB tile per batch ([128 part, 16 rows]), i.e.
    32 KB DMA descriptors -- the DMA engines pay ~125 ns per descriptor of
    dead time, so big descriptors buy ~20% more effective bandwidth.  The 16
    tile-DMAs (one per (batch, channel), 32 partitions each) are split over
    the sync and scalar HW queues and issued up-front; the whole input lives
    in SBUF.
  * output stores are FENCED behind the last input DMA (tiny dependency
    DMAs/copies), then spread over all three DMA queues: during input
    streaming the engine pool runs input only, and the ~12 us of store
    drain overlaps the last batch's compute tail instead of delaying T_in.
  * the t-reduction (16 rows -> 4 per partition) runs on the vector engine
    as two full-width adds per batch; one 128-contraction matmul per j-slot
    against a channel-selector matrix then yields all 4 channel colsums in
    PSUM [4, 512] -- the PE (stuck at its cold 1.2 GHz clock here) streams
    only 4x512 columns per batch for the reduction.
  * stats live q-major in S2 [36, 512] (q*4+ci; q=0 colsum, 1..4 top rows,
    5..8 bottom rows); the R-summary contraction runs over all 36 rows
    against zero-padded per-(ci,dw) constant blocks, so every engine op is
    partition-contiguous.  Operands are swapped so R arrives transposed
    (no PE transpose), and L @ A^T is folded into one host constant; both
    channels' tg come from ONE [14, 512] matmul.
  * output quarters (PSUM f32) are cast to bf16 by the scalar engine and
    stored as bf16 (halves write traffic; ~0.3% rounding vs the 2e-2 gate).
    The host upcasts to f32.

Data parallel over batch: 8 cores x 4 batches each; params replicated.
"""
import numpy as np

H = W = 512
K = 7
CIN = 4
CO = 2
BLOC = 4    # batches per core
NCORES = 8

_PROG = None          # cached Bass program (weight-independent; weights are inputs)
TRACE = False         # set True (e.g. from test.py) to profile; see LAST_EXEC_NS
LAST_EXEC_NS = None
LAST_TRACE_PATH = None


# ---------------------------------------------------------------------------
# host-side constant builders (all tiny, derived from conv weights)
# ---------------------------------------------------------------------------

def resize_mat(in_size, out_size):
    """Bilinear (half-pixel, edge-normalized) interpolation matrix [out, in],
    matching jax.image.resize(method='bilinear') for upsampling."""
    inv_scale = in_size / out_size
    sample_f = (np.arange(out_size) + 0.5) * inv_scale - 0.5
    xw = np.abs(sample_f[None, :] - np.arange(in_size)[:, None])
    weights = np.maximum(0, 1 - xw)
    total = weights.sum(axis=0, keepdims=True)
    return (weights / total).T.astype(np.float32)  # [out, in]


def build_lhsTR(g_w, g_b, theta_w, theta_b):
    """Phase-2 weight blocks (per batch; identical for every b).

    Returns (blk [4, 3, 9, 14], bias [1, 14]):
      blk[ci, dw, q, col]: coefficient of stats row q of channel ci
        (q: 0=colsum over h, 1..4=x rows 0..3, 5..8=x rows 508..511)
        in output row col = co*7 + i -> R[co, i][w] under w-shift dw.
      bias[0, col]: additive constant (applies to every w of R[col]).
    """
    gw = g_w.astype(np.float64)
    gb = g_b.astype(np.float64)
    tw = theta_w.astype(np.float64)[:, :, 0, 0]
    tb = theta_b.astype(np.float64)
    blk = np.zeros((CIN, 3, 9, 14), dtype=np.float64)
    bias = np.zeros((1, 14), dtype=np.float64)

    def add_F(col, co, dw, sign):
        for ci in range(CIN):
            blk[ci, dw, 0, col] += sign * gw[co, ci, :, dw].sum()
            blk[ci, dw, 1, col] += -sign * gw[co, ci, 2, dw]   # x row 0
            blk[ci, dw, 8, col] += -sign * gw[co, ci, 0, dw]   # x row 511
            if dw == 1:
                blk[ci, dw, 0, col] += sign * tw[co, ci]
        if dw == 1:
            bias[0, col] += sign * H * (gb[co] + tb[co])

    def add_bd(col, co, r, dw, sign):
        for ci in range(CIN):
            for dh in range(3):
                hr = r + dh - 1
                if 0 <= hr < H:
                    q = 1 + hr if hr <= 3 else 5 + (hr - (H - 4))
                    blk[ci, dw, q, col] += sign * gw[co, ci, dh, dw]
            if dw == 1:
                q = 1 + r if r <= 3 else 5 + (r - (H - 4))
                blk[ci, dw, q, col] += sign * tw[co, ci]
        if dw == 1:
            bias[0, col] += sign * (gb[co] + tb[co])

    for co in range(CO):
        for i in range(K):
            col = co * 7 + i
            for dw in range(3):
                add_F(col, co, dw, 1.0)
                if i < 3:
                    for r in range(H - 3 + i, H):
                        add_bd(col, co, r, dw, -1.0)
                elif i > 3:
                    for r in range(0, i - 3):
                        add_bd(col, co, r, dw, -1.0)
    return blk.astype(np.float32), bias.astype(np.float32)


def build_L():
    """lhsT [7, 7] mapping R-summary columns to m columns (includes the
    1/(H*W) patch-mean scale).

    Row e' order matches the R-summary columns: 0 -> total sum,
    1..3 -> R[w=0..2], 4..6 -> R[w=509..511].
    Column j yields m[i, j] = T_R - partial edge sums."""
    L = np.zeros((7, 7), dtype=np.float64)
    L[0, :] = 1.0
    for j in range(3):            # j=0,1,2: subtract tail elements w >= 509+j
        for e in range(3 + j, 6):
            L[1 + e, j] = -1.0    # e=3,4,5 -> rows 4..6
    for j in range(4, 7):         # j=4,5,6: subtract head elements w < j-3
        for e in range(0, j - 3):
            L[1 + e, j] = -1.0    # e=0,1,2 -> rows 1..3
    return (L / (H * W)).astype(np.float32)


def build_consts(g_w, g_b, theta_w, theta_b):
    import ml_dtypes
    blk, biasrow = build_lhsTR(g_w, g_b, theta_w, theta_b)
    # Stats split into a boundary part (rows available as soon as the tiny
    # boundary DMAs land -> contracted EARLY, off the critical tail) and a
    # colsum part (3 tiny matmuls after the last tile).
    # blkb[(q-1)*4+ci', ci*3+dw, col] = blk[ci, dw, q, col] for q=1..8 when
    # ci'==ci else 0 (zero-padded so every lhsT is partition-contiguous).
    blkb = np.zeros((32, CIN * 3, 14), dtype=np.float32)
    for ci in range(CIN):
        for dw in range(3):
            for q in range(1, 9):
                blkb[(q - 1) * 4 + ci, ci * 3 + dw, :] = blk[ci, dw, q, :]
    # blkc[ci, dw, col]: colsum-row coefficients; partition IS the channel.
    blkc = np.ascontiguousarray(blk[:, :, 0, :])        # [4, 3, 14]
    A = resize_mat(K, H)          # [512, 7]
    biaspat = np.ones((1, 7), dtype=np.float32)
    biaspat[0, 0] = float(W)      # total-sum column gets bias once per w
    # tg[i, w'] = sum_e Rt[e, co*7+i] * (L @ A^T)[e, w']  -- fold L into the
    # upsample so no on-device transpose / extra matmul stage is needed.
    la = np.ascontiguousarray(build_L() @ A.T)                    # [7, 512]
    # ocol[:, ci, :]: ones in column ci -> the ci-th colsum matmul deposits
    # its [1, 512] result into row ci of the shared st4 PSUM tile (and adds
    # zero to the other rows), so no PSUM partition offsets are needed.
    ocol = np.zeros((128, CIN, CIN), dtype=np.float32)
    for ci in range(CIN):
        ocol[:, ci, ci] = 1.0
    return {
        "blkb": blkb,
        "blkc": blkc,
        "biasrow": biasrow,
        "biaspat": biaspat,
        "la": la,
        "atr": np.ascontiguousarray(
            A.reshape(128, 4, K).transpose(1, 2, 0)).astype(ml_dtypes.bfloat16),
        "ocol": ocol,
    }


# ---------------------------------------------------------------------------
# device program
# ---------------------------------------------------------------------------

def build_program():
    import concourse.bass as bass
    import concourse.bacc as bacc
    import concourse.tile as tile
    from concourse import mybir

    f32 = mybir.dt.float32
    f32r = mybir.dt.float32r
    bf16 = mybir.dt.bfloat16
    nc = bacc.Bacc(None, target_bir_lowering=False, enable_partition_id=False)

    xs = nc.dram_tensor("xs", [BLOC, CIN, H, W], f32r, kind="ExternalInput")
    blkb_d = nc.dram_tensor("blkb", [32, 12, 14], f32r, kind="ExternalInput")
    blkc_d = nc.dram_tensor("blkc", [4, 3, 14], f32r, kind="ExternalInput")
    bias_d = nc.dram_tensor("biasrow", [1, 14], f32r, kind="ExternalInput")
    bpat_d = nc.dram_tensor("biaspat", [1, 7], f32r, kind="ExternalInput")
    la_d = nc.dram_tensor("la", [7, 512], f32r, kind="ExternalInput")
    atr_d = nc.dram_tensor("atr", [4, 7, 128], bf16, kind="ExternalInput")
    ocol_d = nc.dram_tensor("ocol", [128, CIN, CIN], f32r, kind="ExternalInput")
    y = nc.dram_tensor("y", [BLOC, CO, H, W], bf16, kind="ExternalOutput")

    with tile.TileContext(nc) as tc:
        with (
            tc.tile_pool(name="consts", bufs=1) as consts,
            tc.tile_pool(name="xpool", bufs=16) as xpool,
            tc.tile_pool(name="hpool", bufs=2) as hpool,
            tc.tile_pool(name="gpool", bufs=2) as gpool,
            tc.tile_pool(name="qpool", bufs=2) as qpool,
            tc.tile_pool(name="spool", bufs=4) as spool,
            tc.tile_pool(name="vpool", bufs=2) as vpool,
            tc.tile_pool(name="small", bufs=2) as small,
            tc.tile_pool(name="tgpool", bufs=2) as tgpool,
            # all 8 output tiles stay live: stores are fenced until input
            # streaming completes, so no ob buffer can be recycled earlier
            tc.tile_pool(name="obuf", bufs=8) as obuf,
            tc.tile_pool(name="fence", bufs=1) as fpool,
            tc.tile_pool(name="pstats", bufs=1, space="PSUM") as pstats,
            tc.tile_pool(name="pr", bufs=4, space="PSUM") as pr,
            tc.tile_pool(name="ptg", bufs=1, space="PSUM") as ptg,
            tc.tile_pool(name="poc", bufs=2, space="PSUM") as poc,
        ):
            c_ocol = consts.tile([128, CIN, CIN], f32r)
            nc.gpsimd.dma_start(out=c_ocol, in_=ocol_d[:, :, :])

            # ---- issue ALL input DMAs up-front (pure-load phase) ----
            # ALL boundary-row DMAs go first (0.5 MB total): every batch's
            # boundary stats are then computable within the first ~12 us,
            # off the per-batch critical chain.
            # boundary-stats tile: row (q-1)*4+ci (q=1..4 x rows 0..3,
            # q=5..8 x rows 508..511).  PLAIN partition-slice targets (a
            # partition-split output AP here miscompiles); the (r, c)
            # interleave happens on the DRAM side of the transfer.
            svec = []
            for b in range(BLOC):
                S = spool.tile([32, 512], f32r, tag="S")
                nc.sync.dma_start(
                    out=S[0:16, :],
                    in_=xs[b, :, 0:4, :].rearrange("c r w -> r c w"),
                )
                nc.scalar.dma_start(
                    out=S[16:32, :],
                    in_=xs[b, :, 508:512, :].rearrange("c r w -> r c w"),
                )
                svec.append(S)
            # per-(b, ci) tiles with 4 rows per partition: the 8 KB
            # contiguous runs become 8 KB DMA descriptors, the measured
            # sweet spot (~22 GB/s per engine vs ~13 GB/s at 32 KB)
            tiles = []
            for b in range(BLOC):
                row = []
                for ci in range(CIN):
                    xt = xpool.tile([128, 4, 512], f32r, tag="xt")
                    eng = nc.sync if (b + ci) % 2 == 0 else nc.scalar
                    eng.dma_start(
                        out=xt,
                        in_=xs[b, ci].rearrange("(p t) w -> p t w", t=4),
                    )
                    row.append(xt)
                tiles.append(row)

            c_blkb = consts.tile([32, 12, 14], f32r)
            nc.gpsimd.dma_start(out=c_blkb, in_=blkb_d[:, :, :])
            c_blkc = consts.tile([4, 3, 14], f32r)
            nc.gpsimd.dma_start(out=c_blkc, in_=blkc_d[:, :, :])
            c_bias = consts.tile([1, 14], f32r)
            nc.gpsimd.dma_start(out=c_bias, in_=bias_d[:, :])
            c_bpat = consts.tile([1, 7], f32r)
            nc.gpsimd.dma_start(out=c_bpat, in_=bpat_d[:, :])
            c_la = consts.tile([7, 512], f32r)
            nc.gpsimd.dma_start(out=c_la, in_=la_d[:, :])
            c_atr = consts.tile([7, 4, 128], bf16)
            nc.gpsimd.dma_start(out=c_atr, in_=atr_d.rearrange("t j p -> j t p"))

            # store-delay fences (emitted lazily before each queue's FIRST
            # store): a tiny read of the LAST batch's final tiles parks the
            # store queue until input streaming is done, so output stores do
            # not steal DMA-engine time while input is still streaming.
            # Only sync and gpsimd carry stores -- the scalar queue runs the
            # casts and must never block on a fence.
            lsync = tiles[BLOC - 1][3]    # (3+3)%2==0 -> sync queue's last
            lscal = tiles[BLOC - 1][2]    # scalar queue's last input DMA
            ftile = fpool.tile([128, 8], f32r, tag="fence")
            fenced = set()

            def fence_queue(qname):
                if qname in fenced:
                    return
                fenced.add(qname)
                if qname == "gpsimd":
                    nc.gpsimd.tensor_copy(ftile[:, 0:1],
                                          lsync[:, 3, 511:512])
                    nc.gpsimd.tensor_copy(ftile[:, 1:2],
                                          lscal[:, 3, 511:512])
                else:
                    nc.sync.dma_start(out=ftile[127:128, 2:3],
                                      in_=lsync[127:128, 3, 511:512])
                    nc.sync.dma_start(out=ftile[127:128, 3:4],
                                      in_=lscal[127:128, 3, 511:512])

            def stage_v(S, nrows, tag):
                # ---- per-row summaries V = [T | edges], batched DVE ops ----
                # V column groups, one per w-shift dw (7 cols each):
                #  dw=0: [T-S511, 0,  S0, S1, S508, S509, S510]
                #  dw=1: [T,      S0, S1, S2, S509, S510, S511]
                #  dw=2: [T-S0,   S1, S2, S3, S510, S511, 0   ]
                # Works on the [32, 512] boundary tile and on the [4, 512]
                # colsum PSUM tile alike.
                V = vpool.tile([nrows, 21], f32r, tag=tag)
                with nc.allow_low_precision(
                        reason="f32r is f32 storage; single-pass matmul mode"):
                    nc.vector.reduce_sum(V[:, 7:8], S,
                                         axis=mybir.AxisListType.X)
                edges = bass.AP(           # S columns {0,1,2, 509,510,511}
                    tensor=S.tensor, offset=S.offset,
                    ap=[S.ap[0], [509, 2], [1, 3]],
                )
                nc.vector.tensor_copy(
                    V[:, 8:14].rearrange("p (g e) -> p g e", g=2), edges)
                nc.vector.tensor_sub(V[:, 0:1], V[:, 7:8], V[:, 13:14])
                nc.vector.memset(V[:, 1:2].bitcast(f32), 0.0)
                nc.vector.tensor_copy(V[:, 2:4], V[:, 8:10])
                nc.vector.tensor_copy(V[:, 4:7], S[:, 508:511])
                nc.vector.tensor_sub(V[:, 14:15], V[:, 7:8], V[:, 8:9])
                nc.vector.tensor_copy(V[:, 15:18], S[:, 1:4])
                nc.vector.tensor_copy(V[:, 18:20], V[:, 12:14])
                nc.vector.memset(V[:, 20:21].bitcast(f32), 0.0)
                return V

            # ---- FRONT-LOADED boundary work: all batches' Vb summaries and
            # their 12 R matmuls run in the first ~15 us (boundary DMAs are
            # tiny and were issued first) -- only the colsum part of each
            # batch's stats stays on its critical chain.
            rvec = []
            for b in range(BLOC):
                Vb = stage_v(svec[b], 32, "Vb")
                Rt_ps = pr.tile([7, 14], f32, tag="Rt")
                nc.tensor.matmul(Rt_ps, c_bpat, c_bias, start=True, stop=False)
                for ci in range(CIN):
                    for dw in range(3):
                        nc.tensor.matmul(
                            Rt_ps, Vb[:, 7 * dw:7 * dw + 7],
                            c_blkb[:, ci * 3 + dw, :],
                            start=False, stop=False)
                rvec.append(Rt_ps)

            # ---- per-batch compute; the DMA queues are already loaded ----
            stores = []
            for b in range(BLOC):
                Rt_ps = rvec[b]
                # phase 1: per-channel t-reduction 4 -> 1 rows, split DVE /
                # gpsimd (gpsimd adds are ~2x slower: DVE gets ch 0+2 fully
                # plus ch 3's level-2; gpsimd gets ch 1 fully + ch 3 level-1.
                # The st4 chain visits DVE-fed channels first so the PE never
                # head-of-line blocks on a lagging gpsimd add).  One matmul
                # per channel against a ones-in-column-ci lhsT deposits each
                # colsum into row ci of the st4 PSUM tile.
                qx = qpool.tile([128, 4, 512], f32r, tag="qx")
                st4 = pstats.tile([4, 512], f32, tag="st4")
                lvl1 = {0: nc.vector, 2: nc.vector, 1: nc.gpsimd,
                        3: nc.gpsimd}
                lvl2 = {0: nc.vector, 2: nc.vector, 1: nc.gpsimd,
                        3: nc.vector}
                for k, ci in enumerate((0, 2, 1, 3)):
                    xt = tiles[b][ci]
                    pool = hpool if lvl1[ci] is nc.vector else gpool
                    hx = pool.tile([128, 2, 512], f32r, tag="hx")
                    lvl1[ci].tensor_tensor(hx, xt[:, 0:2, :], xt[:, 2:4, :],
                                           op=mybir.AluOpType.add)
                    lvl2[ci].tensor_tensor(qx[:, ci, :], hx[:, 0, :],
                                           hx[:, 1, :],
                                           op=mybir.AluOpType.add)
                    nc.tensor.matmul(st4, c_ocol[:, ci, :], qx[:, ci, :],
                                     start=(k == 0), stop=(k == CIN - 1))

                # LATE half: colsum V summaries straight from PSUM (no SBUF
                # staging copy), then only 3 tiny R matmuls close Rt.
                Vc = stage_v(st4, 4, "Vc")
                for dw in range(3):
                    nc.tensor.matmul(Rt_ps, Vc[:, 7 * dw:7 * dw + 7],
                                     c_blkc[:, dw, :],
                                     start=False, stop=(dw == 2))
                Rt = small.tile([7, 14], f32r, tag="Rtsb")
                nc.scalar.copy(Rt, Rt_ps)

                # phase 3: per-channel tg = R_co @ (L A^T) (PE rhs base
                # partition must be 0/32/64, so no fused [14, 512] tg), then
                # the two channels' upsample ladders interleave, with casts
                # split between the scalar (co 0) and vector (co 1) engines
                # so the tail ladder runs both chains concurrently.
                tgs = []
                for co in range(CO):
                    tg_ps = ptg.tile([7, 512], f32, tag="tg_ps")
                    nc.tensor.matmul(tg_ps, Rt[:, co * 7:co * 7 + 7], c_la,
                                     start=True, stop=True)
                    tg = tgpool.tile([7, 512], bf16, tag="tg")
                    nc.scalar.copy(tg, tg_ps)
                    tgs.append(tg)
                ob0 = obuf.tile([128, 4, 512], bf16, tag="ob")
                ob1 = obuf.tile([128, 4, 512], bf16, tag="ob")
                obs = [ob0, ob1]
                for t in range(4):
                    for co in range(CO):
                        oc_ps = poc.tile([128, 512], f32, tag="oc")
                        nc.tensor.matmul(oc_ps, c_atr[:, t, :], tgs[co],
                                         start=True, stop=True)
                        if co == 0:
                            nc.scalar.copy(obs[co][:, t, :], oc_ps)
                        else:
                            nc.vector.tensor_copy(obs[co][:, t, :], oc_ps)
                for co in range(CO):
                    # stores are collected and emitted after ALL per-batch
                    # compute: the gpsimd queue also runs level-1 adds, and a
                    # fenced store emitted mid-stream would block the next
                    # batch's adds on the in-order queue.
                    stores.append((b, co, obs[co]))

            # ---- store flush: fence both store queues, then alternate ----
            for g, (b, co, ob) in enumerate(stores):
                qname = "sync" if g % 2 == 0 else "gpsimd"
                fence_queue(qname)
                seng = nc.sync if g % 2 == 0 else nc.gpsimd
                seng.dma_start(
                    out=y[b, co].rearrange("(p t) w -> p t w", t=4),
                    in_=ob,
                )
    return nc


def _get_prog():
    global _PROG
    if _PROG is None:
        _PROG = build_program()
        _PROG.finalize()
    return _PROG


# ---------------------------------------------------------------------------
# host entry point
# ---------------------------------------------------------------------------

def kernel(x, g_w, g_b, theta_w, theta_b):
    global LAST_EXEC_NS, LAST_TRACE_PATH
    from concourse.bass_utils import run_bass_kernel_spmd

    x = np.ascontiguousarray(np.asarray(x, dtype=np.float32))
    g_w = np.asarray(g_w, dtype=np.float32)
    g_b = np.asarray(g_b, dtype=np.float32)
    theta_w = np.asarray(theta_w, dtype=np.float32)
    theta_b = np.asarray(theta_b, dtype=np.float32)

    consts = build_consts(g_w, g_b, theta_w, theta_b)
    nc = _get_prog()
    in_maps = [
        {"xs": np.ascontiguousarray(x[c * BLOC:(c + 1) * BLOC]), **consts}
        for c in range(NCORES)
    ]
    res = run_bass_kernel_spmd(nc, in_maps, core_ids=list(range(NCORES)),
                               trace=TRACE)
    LAST_EXEC_NS = res.exec_time_ns
    if TRACE and res.instructions_and_trace is not None:
        LAST_TRACE_PATH = res.instructions_and_trace[1]
    return np.concatenate(
        [np.asarray(res.results[c]["y"]).astype(np.float32)
         for c in range(NCORES)], axis=0)
